# revision 1
# baseline (speedup 1.0000x reference)
"""cosFormer non-causal linear attention on 8 trn2 NeuronCores.

Data-parallel over batch N=8: core b computes batch element b end-to-end.
Per core (L=2048, E=1024, H=16 heads, d=64):
  q = relu(x @ Wq.T), k = relu(x @ Wk.T), v = x @ Wv.T
  q_ = [q*sin, q*cos], k_ = [k*sin, k*cos]    (per-position cos/sin reweight)
  kv_h = k_h^T @ [v_h | 1]                     (accumulated over L in PSUM;
                                                the ones column yields k_sum)
  attn_h^T = kv_h^T @ q_h^T, rows 0:64 scaled by z = 1/row 64
  out = attn @ Wo.T

Layouts: activations contract over E_in, so x and all weights enter
transposed (host-side). k_, v live L-major; q_, attn^T live feature-major,
which feeds the output projection without any transposes on device.
All matmuls bf16 with fp32 PSUM accumulation.

Schedule notes (v4):
- Inputs arrive via a few large DMAs, all issued on the sync queue in
  first-use order so arrival order matches need order; xt comes in
  L-quarters so the first k-projection group only needs wk-half0 +
  xt-quarter0 + wv-half0 (2.5 MB) instead of 6 MB.
- A dozen dummy matmuls on zero tiles run during the DMA wait to hold the
  PE HAM clock-gate open so real matmuls start at 2.4 GHz.
- Phase 2 z-chain (denominator -> reciprocal -> partition-broadcast ->
  scale) is off the PE critical path: each attention PSUM is evacuated by
  one [65,512] ACT copy, so the 4-bank pa rotation never waits on the
  chain.  Denominator rows are assembled by ACT at 32-aligned partitions
  (engine APs require 32-aligned partition bases) so reciprocals batch 4
  heads per DVE op; reciprocal rows are then staged back to partition-0
  tiles because the GpSimd partition_broadcast ucode reads partition 0
  regardless of the AP offset (HW diverges from the simulator here).
- The output projection of chunk lc is emitted a window later
  (attn(lc)+qproj(lc+1) quads -> out(lc-1) -> z-chains(lc)), so chains
  drain under other PE work; attention head-quads interleave with
  q-projection groups so the PE never stalls on the bank rotation, and
  attn(3) interleaves with out(2) so the final window is pure out(3).
- eps is dropped from the denominator: it is a sum of 128 nonnegative
  products and is zero only if relu killed all 64 features of a query
  position (probability ~2^-64).
- Output is written bf16 (host upcasts) to halve the out-DMA bytes.

PSUM plan (8 banks): A(2 bufs): pk/pq (+warmup dummies), B(2): pv/po,
kvp0-3 (1 each): kv accumulators in phase 1, then recycled as attention
psums in phase 2.  kv slices accumulate across all of phase 1 with
start=False (banks pre-cleared by a K=1 zero matmul with stop=False:
start=True clears has_written for the WHOLE bank, so head-slices sharing
a bank never issue their own starts, and only the last head per bank
issues stop).
"""

import numpy as np
import ml_dtypes

import concourse.tile as tile
from concourse import bacc, mybir
from concourse.bass_utils import run_bass_kernel_spmd

L, NB, E, H, D = 2048, 8, 1024, 16, 64
KT = E // 128          # 8 contraction tiles
LC = 4                 # L chunks of 512
LCW = L // LC          # 512
LT = L // 128          # 16 l-tiles
NWARM = 12             # HAM warm-up dummy matmuls

f32 = mybir.dt.float32
bf16 = mybir.dt.bfloat16
AL = mybir.AluOpType
AF = mybir.ActivationFunctionType
BF = np.dtype(ml_dtypes.bfloat16)

_CACHE = {}


def _build():
    if "nc" in _CACHE:
        return _CACHE["nc"]
    nc = bacc.Bacc()

    xt_d = nc.declare_dram_parameter("xt", [E, L], bf16, isOutput=False)
    wq_d = nc.declare_dram_parameter("wq", [E, E], bf16, isOutput=False)
    wk_d = nc.declare_dram_parameter("wk", [E, E], bf16, isOutput=False)
    wv_d = nc.declare_dram_parameter("wv", [E, E], bf16, isOutput=False)
    wo_d = nc.declare_dram_parameter("wo", [E, E], bf16, isOutput=False)
    scol_d = nc.declare_dram_parameter("scol", [128, LT * 2], f32, isOutput=False)
    srow_d = nc.declare_dram_parameter("srow", [128, L], bf16, isOutput=False)
    srow2_d = nc.declare_dram_parameter("srow2", [128, L], bf16, isOutput=False)
    out_d = nc.declare_dram_parameter("out", [L, E], bf16, isOutput=True)

    with tile.TileContext(nc) as tc:
        with (
            tc.tile_pool(name="const", bufs=1) as cp,
            tc.tile_pool(name="work", bufs=1) as wp,
            tc.tile_pool(name="ps", bufs=1, space="PSUM") as pp,
        ):
            # ---- resident constants -------------------------------------
            # Big DMAs, all on the sync queue, in first-use order.
            xt_re = xt_d.rearrange("(t p) l -> p t l", p=128)
            w_re = {n: d.rearrange("(t p) e -> p t e", p=128)
                    for n, d in (("wk", wk_d), ("wv", wv_d), ("wq", wq_d),
                                 ("wo", wo_d))}
            scol = cp.tile([128, LT, 2], f32, tag="scol")
            wkh = [cp.tile([128, KT, 512], bf16, tag=f"wkh{h}", name=f"wkh{h}")
                   for h in range(2)]
            wvh = [cp.tile([128, KT, 512], bf16, tag=f"wvh{h}", name=f"wvh{h}")
                   for h in range(2)]
            xtq = [cp.tile([128, KT, 512], bf16, tag=f"xtq{q}", name=f"xtq{q}")
                   for q in range(4)]
            wqt = cp.tile([128, KT, E], bf16, tag="wqt")
            wot = cp.tile([128, KT, E], bf16, tag="wot")
            # srow rows 0:64 = sin (replicated), rows 64:128 = cos;
            # srow2 is srow with halves swapped (TensorTensor requires
            # equal base partitions when both inputs are in SBUF)
            srow = cp.tile([128, L], bf16, tag="srow")
            srow2 = cp.tile([128, L], bf16, tag="srow2")

            nc.sync.dma_start(scol[:], scol_d.rearrange("p (t c) -> p t c", c=2)[:])
            nc.sync.dma_start(wkh[0][:], w_re["wk"][:, :, 0:512])
            nc.sync.dma_start(xtq[0][:], xt_re[:, :, 0:512])
            nc.sync.dma_start(wvh[0][:], w_re["wv"][:, :, 0:512])
            nc.sync.dma_start(wkh[1][:], w_re["wk"][:, :, 512:1024])
            nc.sync.dma_start(wvh[1][:], w_re["wv"][:, :, 512:1024])
            for q in range(1, 4):
                nc.sync.dma_start(xtq[q][:], xt_re[:, :, q * 512:(q + 1) * 512])
            nc.sync.dma_start(wqt[:], w_re["wq"][:])
            nc.sync.dma_start(wot[:], w_re["wo"][:])
            nc.sync.dma_start(srow[:], srow_d[:])
            nc.sync.dma_start(srow2[:], srow2_d[:])

            # kv lhsT tiles in SBUF: 4 groups of 4 heads, (128=2d, 4, 65)
            kv_sb = [wp.tile([128, 4, 65], bf16, tag=f"kv{g}", name=f"kv{g}")
                     for g in range(4)]

            # ---- psum accumulators for phase 1 --------------------------
            kv_ps = [pp.tile([128, 4, 65], f32, tag=f"kvp{g}", name=f"kvp{g}")
                     for g in range(4)]
            zl = wp.tile([1, 128], bf16, tag="zl")
            zrow = wp.tile([1, 512], bf16, tag="zrow")
            nc.vector.memset(zl[:], 0.0)
            nc.vector.memset(zrow[:], 0.0)
            for g in range(4):
                nc.tensor.matmul(kv_ps[g][:].rearrange("p a c -> p (a c)"),
                                 zl[:], zrow[:, 0:260], start=True, stop=False)
            # HAM warm-up: keep PE busy during the input DMA wait so real
            # matmuls start at the full clock.  Never read.
            dmy = pp.tile([128, 512], f32, tag="A", bufs=2, name="dmy")
            for _ in range(NWARM):
                nc.tensor.matmul(dmy[:], zl[:], zrow[:], start=True, stop=True)

            # ---- phase 1: kv accumulation (kv mms pipelined 1 iter back,
            # so PE has a full projection group between an ACT build and its
            # kv consumers and never stalls on ACT latency) ---------------
            pending = None
            def flush_kv(pend):
                kb_, vb_, lt_, eo_ = pend
                for hh in range(8):
                    h = eo_ * 8 + hh
                    nc.tensor.matmul(kv_ps[h // 4][:, h % 4, :],
                                     kb_[:, hh, :], vb_[:, hh, :],
                                     start=False,
                                     stop=(lt_ == LT - 1 and hh % 4 == 3))
            for lt in range(LT):
                q4, off = divmod(lt, 4)
                lsl = slice(off * 128, off * 128 + 128)
                for eo in range(2):
                    pk = pp.tile([128, 512], f32, tag="A", bufs=2, name=f"pk{lt}_{eo}")
                    pv = pp.tile([128, 512], f32, tag="B", bufs=2, name=f"pv{lt}_{eo}")
                    for k in range(KT):
                        nc.tensor.matmul(pk[:], xtq[q4][:, k, lsl], wkh[eo][:, k, :],
                                         start=(k == 0), stop=(k == KT - 1))
                    if pending is not None:
                        flush_kv(pending)
                    for k in range(KT):
                        nc.tensor.matmul(pv[:], xtq[q4][:, k, lsl], wvh[eo][:, k, :],
                                         start=(k == 0), stop=(k == KT - 1))
                    # k_ build on ACT: per head [0:64]=relu(k)*sin, [64:128]=relu(k)*cos
                    kb = wp.tile([128, 8, 128], bf16, tag="kb", bufs=3,
                                 name=f"kb{lt}_{eo}")
                    pk3 = pk[:].rearrange("p (h e) -> p h e", h=8)
                    nc.scalar.activation(kb[:, :, 0:64], pk3,
                                         AF.Relu, scale=scol[:, lt, 0:1])
                    nc.scalar.activation(kb[:, :, 64:128], pk3,
                                         AF.Relu, scale=scol[:, lt, 1:2])
                    # v copy on ACT into 65-wide layout; ones col on DVE
                    vb = wp.tile([128, 8, 65], bf16, tag="vb", bufs=3,
                                 name=f"vb{lt}_{eo}")
                    nc.scalar.activation(vb[:, :, 0:64],
                                         pv[:].rearrange("p (h e) -> p h e", h=8),
                                         AF.Copy)
                    nc.vector.memset(vb[:, :, 64:65], 1.0)
                    pending = (kb, vb, lt, eo)
            flush_kv(pending)
            for g in range(4):
                nc.vector.tensor_copy(kv_sb[g][:], kv_ps[g][:])

            # ---- phase 2: q, attention, output projection ---------------
            qts = {}    # (lc, h) -> q_ tile
            pans = {}   # (lc, h) -> attention rows 0:64 + denom row 64
            ats = {}    # (lc, m) -> scaled attention lhsT tiles
            # denominators live at partitions {0,32,64,96} x column-group lc%2
            # (chunks lc and lc+1 overlap, so two column sets alternate)
            zq = wp.tile([128, 2, 4, LCW], f32, tag="zq")
            nc.vector.memset(zq[:], 1.0)

            def qp_group(lc, m):
                pq = pp.tile([128, LCW], f32, tag="A", bufs=2,
                             name=f"pq{m}_{lc}")
                for k in range(KT):
                    nc.tensor.matmul(pq[:], wqt[:, k, m * 128:(m + 1) * 128],
                                     xtq[lc][:, k, :],
                                     start=(k == 0), stop=(k == KT - 1))
                qr = wp.tile([128, LCW], bf16, tag="qr", bufs=2,
                             name=f"qr{m}_{lc}")
                nc.scalar.activation(qr[:], pq[:], AF.Relu)
                csl = slice(lc * LCW, (lc + 1) * LCW)
                for j in range(2):
                    h = 2 * m + j
                    q_h = wp.tile([128, LCW], bf16, tag=f"qt{h}",
                                  name=f"qt{h}_{lc}", bufs=2)
                    rows = slice(j * 64, j * 64 + 64)
                    sin_src = (srow if j == 0 else srow2)[rows, csl]
                    cos_src = (srow2 if j == 0 else srow)[rows, csl]
                    nc.vector.tensor_tensor(q_h[0:64, :], qr[rows, :],
                                            sin_src, AL.mult)
                    nc.vector.tensor_tensor(q_h[64:128, :], qr[rows, :],
                                            cos_src, AL.mult)
                    qts[(lc, h)] = q_h

            def attn_quad(lc, g):
                for h in range(4 * g, 4 * g + 4):
                    pa = pp.tile([65, LCW], f32, tag=f"kvp{h % 4}",
                                 name=f"pa{h}_{lc}")
                    nc.tensor.matmul(pa[:], kv_sb[h // 4][:, h % 4, :],
                                     qts[(lc, h)][:], start=True, stop=True)
                    # single prompt evacuation frees the PSUM bank: rows
                    # 0:64 = numerator, row 64 = z-denominator
                    pan = wp.tile([65, LCW], bf16, tag=f"pan{h}", bufs=1,
                                  name=f"pan{h}_{lc}")
                    nc.scalar.activation(pan[:], pa[:], AF.Copy)
                    pans[(lc, h)] = pan
                    # denominator row -> partition 32*(h%4), fp32 for recip
                    nc.scalar.activation(
                        zq[32 * (h % 4):32 * (h % 4) + 1, lc % 2, g, :],
                        pan[64:65, :], AF.Copy)

            def chain_quad(lc, g):
                if g == 0:
                    for m in range(KT):
                        ats[(lc, m)] = wp.tile([128, LCW], bf16, tag=f"at{m}",
                                               name=f"at{m}_{lc}", bufs=2)
                nc.vector.reciprocal_approx_fast(zq[:, lc % 2, g, :],
                                                 zq[:, lc % 2, g, :])
                for h in range(4 * g, 4 * g + 4):
                    # stage the reciprocal row at partition 0: the gpsimd
                    # broadcast ucode reads partition 0 of the tile
                    # regardless of the AP's partition offset
                    zbh = wp.tile([1, LCW], bf16, tag="zbh", bufs=3,
                                  name=f"zbh{h}_{lc}")
                    nc.vector.tensor_copy(
                        zbh[:], zq[32 * (h % 4):32 * (h % 4) + 1, lc % 2, g, :])
                    zbt = wp.tile([64, LCW], bf16, tag="zb", bufs=3,
                                  name=f"zb{h}_{lc}")
                    nc.gpsimd.partition_broadcast(zbt[:], zbh[:])
                    rows = slice((h % 2) * 64, (h % 2) * 64 + 64)
                    nc.vector.tensor_tensor(ats[(lc, h // 2)][rows, :],
                                            pans[(lc, h)][0:64, :], zbt[:],
                                            AL.mult)

            def outp_block(lc, ltl, eo):
                lt = lc * 4 + ltl
                tsl = slice(ltl * 128, (ltl + 1) * 128)
                esl = slice(eo * 512, (eo + 1) * 512)
                po = pp.tile([128, 512], f32, tag="B", bufs=2,
                             name=f"po{lt}_{eo}")
                for m in range(KT):
                    nc.tensor.matmul(po[:], ats[(lc, m)][:, tsl],
                                     wot[:, m, esl],
                                     start=(m == 0), stop=(m == KT - 1))
                ob = wp.tile([128, 512], bf16, tag="ob", bufs=2,
                             name=f"ob{lt}_{eo}")
                nc.scalar.activation(ob[:], po[:], AF.Copy)
                nc.sync.dma_start(out_d[lt * 128:(lt + 1) * 128, esl], ob[:])

            for m in range(KT):
                qp_group(0, m)
            # w0: attn(0) quads + qp(1), then chains(0)
            for g in range(4):
                attn_quad(0, g)
                qp_group(1, 2 * g)
                qp_group(1, 2 * g + 1)
            for g in range(4):
                chain_quad(0, g)
            # w1: attn(1) quads + qp(2), then out(0), then chains(1)
            for g in range(4):
                attn_quad(1, g)
                qp_group(2, 2 * g)
                qp_group(2, 2 * g + 1)
            for ltl in range(4):
                outp_block(0, ltl, 0)
                outp_block(0, ltl, 1)
            for g in range(4):
                chain_quad(1, g)
            # w2: attn(2) quads + qp(3), then out(1), then chains(2)
            for g in range(4):
                attn_quad(2, g)
                qp_group(3, 2 * g)
                qp_group(3, 2 * g + 1)
            for ltl in range(4):
                outp_block(1, ltl, 0)
                outp_block(1, ltl, 1)
            for g in range(4):
                chain_quad(2, g)
            # w3: attn(3) quads interleaved with out(2), chains(3) inline
            for g in range(4):
                attn_quad(3, g)
                outp_block(2, g, 0)
                outp_block(2, g, 1)
                chain_quad(3, g)
            # w4: pure out(3)
            for ltl in range(4):
                outp_block(3, ltl, 0)
                outp_block(3, ltl, 1)

    nc.compile()
    _CACHE["nc"] = nc
    return nc


def _prep_inputs(query, Wq, Wk, Wv, Wo):
    idx = (np.pi / 2) * np.arange(1, L + 1, dtype=np.float64) / L
    sin = np.sin(idx).astype(np.float32)
    cos = np.cos(idx).astype(np.float32)
    # scol[p, t, c]: c=0 sin, c=1 cos at l = t*128+p
    scol = np.stack([sin.reshape(LT, 128).T, cos.reshape(LT, 128).T],
                    axis=2).reshape(128, LT * 2).copy()
    srow = np.concatenate([np.tile(sin[None, :], (64, 1)),
                           np.tile(cos[None, :], (64, 1))], axis=0).astype(BF)
    srow2 = np.concatenate([srow[64:128], srow[0:64]]).copy()

    ws = {n: np.ascontiguousarray(w.T).astype(BF)
          for n, w in (("wq", Wq), ("wk", Wk), ("wv", Wv), ("wo", Wo))}
    in_maps = []
    for b in range(NB):
        m = dict(ws)
        m["xt"] = np.ascontiguousarray(query[:, b, :].T).astype(BF)
        m["scol"] = scol
        m["srow"] = srow
        m["srow2"] = srow2
        in_maps.append(m)
    return in_maps


def kernel(query, Wq, Wk, Wv, Wo, _trace=False, _trace_kwargs=None):
    nc = _build()
    in_maps = _prep_inputs(np.asarray(query, np.float32), Wq, Wk, Wv, Wo)
    res = run_bass_kernel_spmd(nc, in_maps, list(range(NB)), trace=_trace,
                               **(_trace_kwargs or {}))
    out = np.stack([np.asarray(res.results[b]["out"]) for b in range(NB)],
                   axis=1)
    if _trace:
        kernel.last_result = res
    return np.ascontiguousarray(out.astype(np.float32))



# revision 11
# speedup vs baseline: 1.0628x; 1.0628x over previous
"""cosFormer non-causal linear attention on 8 trn2 NeuronCores.

Data-parallel over batch N=8: core b computes batch element b end-to-end.
Per core (L=2048, E=1024, H=16 heads, d=64):
  q = relu(x @ Wq.T), k = relu(x @ Wk.T), v = x @ Wv.T
  q_ = [q*sin, q*cos], k_ = [k*sin, k*cos]    (per-position cos/sin reweight)
  kv_h = k_h^T @ [v_h | 1]                     (accumulated over L in PSUM;
                                                the ones column yields k_sum)
  attn_h^T = kv_h^T @ q_h^T, rows 0:64 scaled by z = 1/row 64
  out = attn @ Wo.T

Layouts: activations contract over E_in, so x and all weights enter
transposed (host-side). k_, v live L-major; q_, attn^T live feature-major,
which feeds the output projection without any transposes on device.
All matmuls bf16 with fp32 PSUM accumulation.

Schedule notes (v4):
- Inputs arrive via a few large DMAs, all issued on the sync queue in
  first-use order so arrival order matches need order; xt comes in
  L-quarters so the first k-projection group only needs wk-half0 +
  xt-quarter0 + wv-half0 (2.5 MB) instead of 6 MB.
- A dozen dummy matmuls on zero tiles run during the DMA wait to hold the
  PE HAM clock-gate open so real matmuls start at 2.4 GHz.
- Phase 2 z-chain (denominator -> reciprocal -> partition-broadcast ->
  scale) is off the PE critical path: each attention PSUM is evacuated by
  one [65,512] ACT copy, so the 4-bank pa rotation never waits on the
  chain.  Denominator rows are assembled by ACT at 32-aligned partitions
  (engine APs require 32-aligned partition bases) so reciprocals batch 4
  heads per DVE op; reciprocal rows are then staged back to partition-0
  tiles because the GpSimd partition_broadcast ucode reads partition 0
  regardless of the AP offset (HW diverges from the simulator here).
- The output projection of chunk lc is emitted a window later
  (attn(lc)+qproj(lc+1) quads -> out(lc-1) -> z-chains(lc)), so chains
  drain under other PE work; attention head-quads interleave with
  q-projection groups so the PE never stalls on the bank rotation, and
  attn(3) interleaves with out(2) so the final window is pure out(3).
- eps is dropped from the denominator: it is a sum of 128 nonnegative
  products and is zero only if relu killed all 64 features of a query
  position (probability ~2^-64).
- Output is written bf16 (host upcasts) to halve the out-DMA bytes.

PSUM plan (8 banks): A(2 bufs): pk/pq (+warmup dummies), B(2): pv/po,
kvp0-3 (1 each): kv accumulators in phase 1, then recycled as attention
psums in phase 2.  kv slices accumulate across all of phase 1 with
start=False (banks pre-cleared by a K=1 zero matmul with stop=False:
start=True clears has_written for the WHOLE bank, so head-slices sharing
a bank never issue their own starts, and only the last head per bank
issues stop).
"""

import numpy as np
import ml_dtypes

import concourse.tile as tile
from concourse import bacc, bass, mybir
from concourse.bass_utils import run_bass_kernel_spmd

L, NB, E, H, D = 2048, 8, 1024, 16, 64
KT = E // 128          # 8 contraction tiles
LC = 4                 # L chunks of 512
LCW = L // LC          # 512
LT = L // 128          # 16 l-tiles
NWARM = 12             # HAM warm-up dummy matmuls

f32 = mybir.dt.float32
bf16 = mybir.dt.bfloat16
AL = mybir.AluOpType
AF = mybir.ActivationFunctionType
BF = np.dtype(ml_dtypes.bfloat16)

_CACHE = {}


def _build():
    if "nc" in _CACHE:
        return _CACHE["nc"]
    nc = bacc.Bacc()

    xt_d = nc.declare_dram_parameter("xt", [E, L], bf16, isOutput=False)
    wq_d = nc.declare_dram_parameter("wq", [E, E], bf16, isOutput=False)
    wk_d = nc.declare_dram_parameter("wk", [E, E], bf16, isOutput=False)
    wv_d = nc.declare_dram_parameter("wv", [E, E], bf16, isOutput=False)
    wo_d = nc.declare_dram_parameter("wo", [E, E], bf16, isOutput=False)
    scol_d = nc.declare_dram_parameter("scol", [128, LT * 2], f32, isOutput=False)
    srow_d = nc.declare_dram_parameter("srow", [128, L], bf16, isOutput=False)
    srow2_d = nc.declare_dram_parameter("srow2", [128, L], bf16, isOutput=False)
    out_d = nc.declare_dram_parameter("out", [L, E], bf16, isOutput=True)

    with tile.TileContext(nc) as tc:
        with (
            tc.tile_pool(name="const", bufs=1) as cp,
            tc.tile_pool(name="work", bufs=1) as wp,
            tc.tile_pool(name="ps", bufs=1, space="PSUM") as pp,
            tc.tile_pool(name="dscratch", bufs=1, space="DRAM") as dp,
        ):
            # ---- resident constants -------------------------------------
            # Big DMAs, all on the sync queue, in first-use order.
            xt_re = xt_d.rearrange("(t p) l -> p t l", p=128)
            w_re = {n: d.rearrange("(t p) e -> p t e", p=128)
                    for n, d in (("wk", wk_d), ("wv", wv_d), ("wq", wq_d),
                                 ("wo", wo_d))}
            scol = cp.tile([128, LT, 2], f32, tag="scol")
            wkh = [cp.tile([128, KT, 512], bf16, tag=f"wkh{h}", name=f"wkh{h}")
                   for h in range(2)]
            wvh = [cp.tile([128, KT, 512], bf16, tag=f"wvh{h}", name=f"wvh{h}")
                   for h in range(2)]
            xtq = [cp.tile([128, KT, 512], bf16, tag=f"xtq{q}", name=f"xtq{q}")
                   for q in range(4)]
            wqt = cp.tile([128, KT, E], bf16, tag="wqt")
            wot = cp.tile([128, KT, E], bf16, tag="wot")
            # srow rows 0:64 = sin (replicated), rows 64:128 = cos;
            # srow2 is srow with halves swapped (TensorTensor requires
            # equal base partitions when both inputs are in SBUF)
            srow = cp.tile([128, L], bf16, tag="srow")
            srow2 = cp.tile([128, L], bf16, tag="srow2")

            nc.sync.dma_start(scol[:], scol_d.rearrange("p (t c) -> p t c", c=2)[:])
            nc.sync.dma_start(wkh[0][:], w_re["wk"][:, :, 0:512])
            nc.sync.dma_start(xtq[0][:], xt_re[:, :, 0:512])
            nc.sync.dma_start(wvh[0][:], w_re["wv"][:, :, 0:512])
            nc.sync.dma_start(wkh[1][:], w_re["wk"][:, :, 512:1024])
            nc.sync.dma_start(wvh[1][:], w_re["wv"][:, :, 512:1024])
            for q in range(1, 4):
                nc.sync.dma_start(xtq[q][:], xt_re[:, :, q * 512:(q + 1) * 512])
            nc.sync.dma_start(wqt[:], w_re["wq"][:])
            nc.sync.dma_start(wot[:], w_re["wo"][:])
            nc.sync.dma_start(srow[:], srow_d[:])
            nc.sync.dma_start(srow2[:], srow2_d[:])

            # kv lhsT tiles in SBUF: 4 groups of 4 heads, (128=2d, 4, 65)
            kv_sb = [wp.tile([128, 4, 65], bf16, tag=f"kv{g}", name=f"kv{g}")
                     for g in range(4)]

            # ---- psum accumulators for phase 1 --------------------------
            kv_ps = [pp.tile([128, 4, 65], f32, tag=f"kvp{g}", name=f"kvp{g}")
                     for g in range(4)]
            zl = wp.tile([1, 128], bf16, tag="zl")
            zrow = wp.tile([1, 512], bf16, tag="zrow")
            nc.vector.memset(zl[:], 0.0)
            nc.vector.memset(zrow[:], 0.0)
            for g in range(4):
                nc.tensor.matmul(kv_ps[g][:].rearrange("p a c -> p (a c)"),
                                 zl[:], zrow[:, 0:260], start=True, stop=False)
            # HAM warm-up: keep PE busy during the input DMA wait so real
            # matmuls start at the full clock.  Never read.
            dmy = pp.tile([128, 512], f32, tag="A", bufs=2, name="dmy")
            for _ in range(NWARM):
                nc.tensor.matmul(dmy[:], zl[:], zrow[:], start=True, stop=True)

            # ---- phase 1: kv accumulation (kv mms pipelined 1 iter back,
            # so PE has a full projection group between an ACT build and its
            # kv consumers and never stalls on ACT latency) ---------------
            pending = None
            def flush_kv(pend):
                kb_, vb_, lt_, eo_ = pend
                for hh in range(8):
                    h = eo_ * 8 + hh
                    nc.tensor.matmul(kv_ps[h // 4][:, h % 4, :],
                                     kb_[:, hh, :], vb_[:, hh, :],
                                     start=False,
                                     stop=(lt_ == LT - 1 and hh % 4 == 3))
            for lt in range(LT):
                q4, off = divmod(lt, 4)
                lsl = slice(off * 128, off * 128 + 128)
                for eo in range(2):
                    pk = pp.tile([128, 512], f32, tag="A", bufs=2, name=f"pk{lt}_{eo}")
                    pv = pp.tile([128, 512], f32, tag="B", bufs=2, name=f"pv{lt}_{eo}")
                    for k in range(KT):
                        nc.tensor.matmul(pk[:], xtq[q4][:, k, lsl], wkh[eo][:, k, :],
                                         start=(k == 0), stop=(k == KT - 1))
                    if pending is not None:
                        flush_kv(pending)
                    for k in range(KT):
                        nc.tensor.matmul(pv[:], xtq[q4][:, k, lsl], wvh[eo][:, k, :],
                                         start=(k == 0), stop=(k == KT - 1))
                    # k_ build on ACT: per head [0:64]=relu(k)*sin, [64:128]=relu(k)*cos
                    kb = wp.tile([128, 8, 128], bf16, tag="kb", bufs=3,
                                 name=f"kb{lt}_{eo}")
                    pk3 = pk[:].rearrange("p (h e) -> p h e", h=8)
                    nc.scalar.activation(kb[:, :, 0:64], pk3,
                                         AF.Relu, scale=scol[:, lt, 0:1])
                    nc.scalar.activation(kb[:, :, 64:128], pk3,
                                         AF.Relu, scale=scol[:, lt, 1:2])
                    # v copy on ACT into 65-wide layout; ones col on DVE
                    vb = wp.tile([128, 8, 65], bf16, tag="vb", bufs=3,
                                 name=f"vb{lt}_{eo}")
                    nc.scalar.activation(vb[:, :, 0:64],
                                         pv[:].rearrange("p (h e) -> p h e", h=8),
                                         AF.Copy)
                    nc.vector.memset(vb[:, :, 64:65], 1.0)
                    pending = (kb, vb, lt, eo)
            flush_kv(pending)
            for g in range(4):
                nc.vector.tensor_copy(kv_sb[g][:], kv_ps[g][:])

            # ---- phase 2: q, attention, output projection ---------------
            qts = {}    # (lc, h) -> q_ tile
            pans = {}   # (lc, h) -> attention rows 0:64 + denom row 64
            zbts = {}   # (lc, h) -> reciprocal denominator broadcast tiles
            ats = {}    # (lc, m) -> scaled attention lhsT tiles
            # denominators live at partitions {0,32,64,96}; the zq/zqb
            # lifetime is intra-quad now (copy -> recip -> cast -> bcast)
            # so two column sets alternating on quad parity suffice.
            # zq is fp32 for the batched reciprocal, zqb its bf16 cast
            zq = wp.tile([128, 2, LCW], f32, tag="zq")
            zqb = wp.tile([128, 2, LCW], bf16, tag="zqb")
            nc.vector.memset(zq[:], 1.0)

            def qp_group(lc, m):
                pq = pp.tile([128, LCW], f32, tag="A", bufs=2,
                             name=f"pq{m}_{lc}")
                for k in range(KT):
                    nc.tensor.matmul(pq[:], wqt[:, k, m * 128:(m + 1) * 128],
                                     xtq[lc][:, k, :],
                                     start=(k == 0), stop=(k == KT - 1))
                qr = wp.tile([128, LCW], bf16, tag="qr", bufs=2,
                             name=f"qr{m}_{lc}")
                nc.scalar.activation(qr[:], pq[:], AF.Relu)
                csl = slice(lc * LCW, (lc + 1) * LCW)
                for j in range(2):
                    h = 2 * m + j
                    q_h = wp.tile([128, LCW], bf16, tag=f"qt{h}",
                                  name=f"qt{h}_{lc}", bufs=2)
                    rows = slice(j * 64, j * 64 + 64)
                    sin_src = (srow if j == 0 else srow2)[rows, csl]
                    cos_src = (srow2 if j == 0 else srow)[rows, csl]
                    nc.vector.tensor_tensor(q_h[0:64, :], qr[rows, :],
                                            sin_src, AL.mult)
                    nc.vector.tensor_tensor(q_h[64:128, :], qr[rows, :],
                                            cos_src, AL.mult)
                    qts[(lc, h)] = q_h

            def attn_quad(lc, g):
                for h in range(4 * g, 4 * g + 4):
                    pa = pp.tile([65, LCW], f32, tag=f"kvp{h % 4}",
                                 name=f"pa{h}_{lc}")
                    nc.tensor.matmul(pa[:], kv_sb[h // 4][:, h % 4, :],
                                     qts[(lc, h)][:], start=True, stop=True)
                    # prompt evacuation frees the PSUM bank: rows 0:64 =
                    # numerator (ACT copy); denominator row 64 -> partition
                    # 32*(h%4) of the fp32 zq quad block (engine APs need
                    # 32-aligned partition bases)
                    pan = wp.tile([65, LCW], bf16, tag=f"pan{h}", bufs=1,
                                  name=f"pan{h}_{lc}")
                    nc.scalar.activation(pan[0:64, :], pa[0:64, :], AF.Copy)
                    pans[(lc, h)] = pan
                    nc.scalar.activation(
                        zq[32 * (h % 4):32 * (h % 4) + 1, g % 2, :],
                        pa[64:65, :], AF.Copy)
                # batched reciprocal for the quad, cast to bf16, then
                # broadcast each head's row to 64 partitions with stride-0
                # DMAs on the otherwise-idle gpsimd queue
                nc.vector.reciprocal_approx_fast(zq[:, g % 2, :],
                                                 zq[:, g % 2, :])
                nc.vector.tensor_copy(zqb[:, g % 2, :], zq[:, g % 2, :])
                for h in range(4 * g, 4 * g + 4):
                    # step-0 partition APs are illegal on SBUF sources, but
                    # legal on DRAM: bounce the 1KB row to DRAM scratch and
                    # read it back broadcast.  Both DMAs ride the gpsimd
                    # queue; Tile tracks the DRAM tile so the read waits.
                    zd = dp.tile([1, LCW], bf16, tag=f"zd{h % 4}", bufs=2,
                                 name=f"zd{h}_{lc}")
                    nc.gpsimd.dma_start(
                        zd[:], zqb[32 * (h % 4):32 * (h % 4) + 1, g % 2, :])
                    zbt = wp.tile([64, LCW], bf16, tag=f"zb{h % 8}", bufs=2,
                                  name=f"zb{h}_{lc}")
                    src = zd[:]
                    src = bass.AP(tensor=src.tensor, offset=src.offset,
                                  ap=[[0, 64], [1, LCW]])
                    nc.gpsimd.dma_start(zbt[:], src)
                    zbts[(lc, h)] = zbt

            def chain_quad(lc, g):
                if g == 0:
                    for m in range(KT):
                        ats[(lc, m)] = wp.tile([128, LCW], bf16, tag=f"at{m}",
                                               name=f"at{m}_{lc}", bufs=2)
                for h in range(4 * g, 4 * g + 4):
                    rows = slice((h % 2) * 64, (h % 2) * 64 + 64)
                    nc.vector.tensor_tensor(ats[(lc, h // 2)][rows, :],
                                            pans[(lc, h)][0:64, :],
                                            zbts[(lc, h)][:], AL.mult)

            def outp_block(lc, ltl, eo):
                lt = lc * 4 + ltl
                tsl = slice(ltl * 128, (ltl + 1) * 128)
                esl = slice(eo * 512, (eo + 1) * 512)
                po = pp.tile([128, 512], f32, tag="B", bufs=2,
                             name=f"po{lt}_{eo}")
                for m in range(KT):
                    nc.tensor.matmul(po[:], ats[(lc, m)][:, tsl],
                                     wot[:, m, esl],
                                     start=(m == 0), stop=(m == KT - 1))
                ob = wp.tile([128, 512], bf16, tag="ob", bufs=2,
                             name=f"ob{lt}_{eo}")
                nc.scalar.activation(ob[:], po[:], AF.Copy)
                nc.sync.dma_start(out_d[lt * 128:(lt + 1) * 128, esl], ob[:])

            for m in range(KT):
                qp_group(0, m)
            # w0: attn(0) quads + qp(1), then chains(0)
            for g in range(4):
                attn_quad(0, g)
                qp_group(1, 2 * g)
                qp_group(1, 2 * g + 1)
            for g in range(4):
                chain_quad(0, g)
            # w1: attn(1) quads + qp(2), then out(0), then chains(1)
            for g in range(4):
                attn_quad(1, g)
                qp_group(2, 2 * g)
                qp_group(2, 2 * g + 1)
            for ltl in range(4):
                outp_block(0, ltl, 0)
                outp_block(0, ltl, 1)
            for g in range(4):
                chain_quad(1, g)
            # w2: attn(2) quads + qp(3), then out(1), then chains(2)
            for g in range(4):
                attn_quad(2, g)
                qp_group(3, 2 * g)
                qp_group(3, 2 * g + 1)
            for ltl in range(4):
                outp_block(1, ltl, 0)
                outp_block(1, ltl, 1)
            for g in range(4):
                chain_quad(2, g)
            # w3: attn(3) quads interleaved with out(2), chains(3) inline
            for g in range(4):
                attn_quad(3, g)
                outp_block(2, g, 0)
                outp_block(2, g, 1)
                chain_quad(3, g)
            # w4: pure out(3)
            for ltl in range(4):
                outp_block(3, ltl, 0)
                outp_block(3, ltl, 1)

    nc.compile()
    _CACHE["nc"] = nc
    return nc


def _prep_inputs(query, Wq, Wk, Wv, Wo):
    idx = (np.pi / 2) * np.arange(1, L + 1, dtype=np.float64) / L
    sin = np.sin(idx).astype(np.float32)
    cos = np.cos(idx).astype(np.float32)
    # scol[p, t, c]: c=0 sin, c=1 cos at l = t*128+p
    scol = np.stack([sin.reshape(LT, 128).T, cos.reshape(LT, 128).T],
                    axis=2).reshape(128, LT * 2).copy()
    srow = np.concatenate([np.tile(sin[None, :], (64, 1)),
                           np.tile(cos[None, :], (64, 1))], axis=0).astype(BF)
    srow2 = np.concatenate([srow[64:128], srow[0:64]]).copy()

    ws = {n: np.ascontiguousarray(w.T).astype(BF)
          for n, w in (("wq", Wq), ("wk", Wk), ("wv", Wv), ("wo", Wo))}
    in_maps = []
    for b in range(NB):
        m = dict(ws)
        m["xt"] = np.ascontiguousarray(query[:, b, :].T).astype(BF)
        m["scol"] = scol
        m["srow"] = srow
        m["srow2"] = srow2
        in_maps.append(m)
    return in_maps


def kernel(query, Wq, Wk, Wv, Wo, _trace=False, _trace_kwargs=None):
    nc = _build()
    in_maps = _prep_inputs(np.asarray(query, np.float32), Wq, Wk, Wv, Wo)
    res = run_bass_kernel_spmd(nc, in_maps, list(range(NB)), trace=_trace,
                               **(_trace_kwargs or {}))
    out = np.stack([np.asarray(res.results[b]["out"]) for b in range(NB)],
                   axis=1)
    if _trace:
        kernel.last_result = res
    return np.ascontiguousarray(out.astype(np.float32))



# revision 15
# speedup vs baseline: 1.0858x; 1.0216x over previous
"""cosFormer non-causal linear attention on 8 trn2 NeuronCores.

Data-parallel over batch N=8: core b computes batch element b end-to-end.
Per core (L=2048, E=1024, H=16 heads, d=64):
  q = relu(x @ Wq.T), k = relu(x @ Wk.T), v = x @ Wv.T
  q_ = [q*sin, q*cos], k_ = [k*sin, k*cos]    (per-position cos/sin reweight)
  kv_h = k_h^T @ [v_h | 1]                     (accumulated over L in PSUM;
                                                the ones column yields k_sum)
  attn_h^T = kv_h^T @ q_h^T, rows 0:64 scaled by z = 1/row 64
  out = attn @ Wo.T

Layouts: activations contract over E_in, so x and all weights enter
transposed (host-side). k_, v live L-major; q_, attn^T live feature-major,
which feeds the output projection without any transposes on device.
All matmuls bf16 with fp32 PSUM accumulation.

Schedule notes (v4):
- Inputs arrive via a few large DMAs, all issued on the sync queue in
  first-use order so arrival order matches need order; xt comes in
  L-quarters so the first k-projection group only needs wk-half0 +
  xt-quarter0 + wv-half0 (2.5 MB) instead of 6 MB.
- A dozen dummy matmuls on zero tiles run during the DMA wait to hold the
  PE HAM clock-gate open so real matmuls start at 2.4 GHz.
- Phase 2 z-chain (denominator -> reciprocal -> partition-broadcast ->
  scale) is off the PE critical path: each attention PSUM is evacuated by
  one [65,512] ACT copy, so the 4-bank pa rotation never waits on the
  chain.  Denominator rows are assembled by ACT at 32-aligned partitions
  (engine APs require 32-aligned partition bases) so reciprocals batch 4
  heads per DVE op; reciprocal rows are then staged back to partition-0
  tiles because the GpSimd partition_broadcast ucode reads partition 0
  regardless of the AP offset (HW diverges from the simulator here).
- The output projection of chunk lc is emitted a window later
  (attn(lc)+qproj(lc+1) quads -> out(lc-1) -> z-chains(lc)), so chains
  drain under other PE work; attention head-quads interleave with
  q-projection groups so the PE never stalls on the bank rotation, and
  attn(3) interleaves with out(2) so the final window is pure out(3).
- eps is dropped from the denominator: it is a sum of 128 nonnegative
  products and is zero only if relu killed all 64 features of a query
  position (probability ~2^-64).
- Output is written bf16 (host upcasts) to halve the out-DMA bytes.

PSUM plan (8 banks): A(2 bufs): pk/pq (+warmup dummies), B(2): pv/po,
kvp0-3 (1 each): kv accumulators in phase 1, then recycled as attention
psums in phase 2.  kv slices accumulate across all of phase 1 with
start=False (banks pre-cleared by a K=1 zero matmul with stop=False:
start=True clears has_written for the WHOLE bank, so head-slices sharing
a bank never issue their own starts, and only the last head per bank
issues stop).
"""

import numpy as np
import ml_dtypes

import concourse.tile as tile
from concourse import bacc, bass, mybir
from concourse.bass_utils import run_bass_kernel_spmd

L, NB, E, H, D = 2048, 8, 1024, 16, 64
KT = E // 128          # 8 contraction tiles
LC = 4                 # L chunks of 512
LCW = L // LC          # 512
LT = L // 128          # 16 l-tiles
NWARM = 16             # HAM warm-up dummy matmuls

f32 = mybir.dt.float32
bf16 = mybir.dt.bfloat16
AL = mybir.AluOpType
AF = mybir.ActivationFunctionType
BF = np.dtype(ml_dtypes.bfloat16)

_CACHE = {}


def _build():
    if "nc" in _CACHE:
        return _CACHE["nc"]
    nc = bacc.Bacc()

    xt_d = nc.declare_dram_parameter("xt", [E, L], bf16, isOutput=False)
    wq_d = nc.declare_dram_parameter("wq", [E, E], bf16, isOutput=False)
    wk_d = nc.declare_dram_parameter("wk", [E, E], bf16, isOutput=False)
    wv_d = nc.declare_dram_parameter("wv", [E, E], bf16, isOutput=False)
    wo_d = nc.declare_dram_parameter("wo", [E, E], bf16, isOutput=False)
    scol_d = nc.declare_dram_parameter("scol", [128, LT * 2], f32, isOutput=False)
    srow_d = nc.declare_dram_parameter("srow", [128, L], bf16, isOutput=False)
    srow2_d = nc.declare_dram_parameter("srow2", [128, L], bf16, isOutput=False)
    out_d = nc.declare_dram_parameter("out", [L, E], bf16, isOutput=True)

    with tile.TileContext(nc) as tc:
        with (
            tc.tile_pool(name="const", bufs=1) as cp,
            tc.tile_pool(name="work", bufs=1) as wp,
            tc.tile_pool(name="ps", bufs=1, space="PSUM") as pp,
            tc.tile_pool(name="dscratch", bufs=1, space="DRAM") as dp,
        ):
            # ---- resident constants -------------------------------------
            # Big DMAs, all on the sync queue, in first-use order.
            xt_re = xt_d.rearrange("(t p) l -> p t l", p=128)
            w_re = {n: d.rearrange("(t p) e -> p t e", p=128)
                    for n, d in (("wk", wk_d), ("wv", wv_d), ("wq", wq_d),
                                 ("wo", wo_d))}
            scol = cp.tile([128, LT, 2], f32, tag="scol")
            wkh = [cp.tile([128, KT, 512], bf16, tag=f"wkh{h}", name=f"wkh{h}")
                   for h in range(2)]
            wvh = [cp.tile([128, KT, 512], bf16, tag=f"wvh{h}", name=f"wvh{h}")
                   for h in range(2)]
            xtq = [cp.tile([128, KT, 512], bf16, tag=f"xtq{q}", name=f"xtq{q}")
                   for q in range(4)]
            wqt = cp.tile([128, KT, E], bf16, tag="wqt")
            wot = cp.tile([128, KT, E], bf16, tag="wot")
            # srow rows 0:64 = sin (replicated), rows 64:128 = cos;
            # srow2 is srow with halves swapped (TensorTensor requires
            # equal base partitions when both inputs are in SBUF)
            srow = cp.tile([128, L], bf16, tag="srow")
            srow2 = cp.tile([128, L], bf16, tag="srow2")

            # first-need tensors split across the two HWDGE rings: the
            # scalar (qActDynamicHW) queue clears its startup barrier ~2us
            # before sync does, and the two rings issue in parallel
            nc.scalar.dma_start(scol[:], scol_d.rearrange("p (t c) -> p t c", c=2)[:])
            nc.scalar.dma_start(wkh[0][:], w_re["wk"][:, :, 0:512])
            nc.scalar.dma_start(wvh[0][:], w_re["wv"][:, :, 0:512])
            nc.sync.dma_start(xtq[0][:], xt_re[:, :, 0:512])
            nc.sync.dma_start(wkh[1][:], w_re["wk"][:, :, 512:1024])
            nc.sync.dma_start(wvh[1][:], w_re["wv"][:, :, 512:1024])
            for q in range(1, 4):
                nc.sync.dma_start(xtq[q][:], xt_re[:, :, q * 512:(q + 1) * 512])
            nc.sync.dma_start(wqt[:], w_re["wq"][:])
            nc.sync.dma_start(wot[:], w_re["wo"][:])
            nc.sync.dma_start(srow[:], srow_d[:])
            nc.sync.dma_start(srow2[:], srow2_d[:])

            # kv lhsT tiles in SBUF: 4 groups of 4 heads, (128=2d, 4, 65)
            kv_sb = [wp.tile([128, 4, 65], bf16, tag=f"kv{g}", name=f"kv{g}")
                     for g in range(4)]

            # ---- psum accumulators for phase 1 --------------------------
            kv_ps = [pp.tile([128, 4, 65], f32, tag=f"kvp{g}", name=f"kvp{g}")
                     for g in range(4)]
            zl = wp.tile([1, 128], bf16, tag="zl")
            zrow = wp.tile([1, 512], bf16, tag="zrow")
            nc.vector.memset(zl[:], 0.0)
            nc.vector.memset(zrow[:], 0.0)
            for g in range(4):
                nc.tensor.matmul(kv_ps[g][:].rearrange("p a c -> p (a c)"),
                                 zl[:], zrow[:, 0:260], start=True, stop=False)
            # HAM warm-up: keep PE busy during the input DMA wait so real
            # matmuls start at the full clock.  Never read.
            dmy = pp.tile([128, 512], f32, tag="A", bufs=2, name="dmy")
            for _ in range(NWARM):
                nc.tensor.matmul(dmy[:], zl[:], zrow[:], start=True, stop=True)

            # ---- phase 1: kv accumulation (kv mms pipelined 1 iter back,
            # so PE has a full projection group between an ACT build and its
            # kv consumers and never stalls on ACT latency) ---------------
            pending = None
            def flush_kv(pend):
                kb_, vb_, lt_, eo_ = pend
                for hh in range(8):
                    h = eo_ * 8 + hh
                    nc.tensor.matmul(kv_ps[h // 4][:, h % 4, :],
                                     kb_[:, hh, :], vb_[:, hh, :],
                                     start=False,
                                     stop=(lt_ == LT - 1 and hh % 4 == 3))
            for lt in range(LT):
                q4, off = divmod(lt, 4)
                lsl = slice(off * 128, off * 128 + 128)
                for eo in range(2):
                    pk = pp.tile([128, 512], f32, tag="A", bufs=2, name=f"pk{lt}_{eo}")
                    pv = pp.tile([128, 512], f32, tag="B", bufs=2, name=f"pv{lt}_{eo}")
                    for k in range(KT):
                        nc.tensor.matmul(pk[:], xtq[q4][:, k, lsl], wkh[eo][:, k, :],
                                         start=(k == 0), stop=(k == KT - 1))
                    if pending is not None:
                        flush_kv(pending)
                    for k in range(KT):
                        nc.tensor.matmul(pv[:], xtq[q4][:, k, lsl], wvh[eo][:, k, :],
                                         start=(k == 0), stop=(k == KT - 1))
                    # k_ build on ACT: per head [0:64]=relu(k)*sin, [64:128]=relu(k)*cos
                    kb = wp.tile([128, 8, 128], bf16, tag="kb", bufs=3,
                                 name=f"kb{lt}_{eo}")
                    pk3 = pk[:].rearrange("p (h e) -> p h e", h=8)
                    nc.scalar.activation(kb[:, :, 0:64], pk3,
                                         AF.Relu, scale=scol[:, lt, 0:1])
                    nc.scalar.activation(kb[:, :, 64:128], pk3,
                                         AF.Relu, scale=scol[:, lt, 1:2])
                    # v copy on ACT into 65-wide layout; ones col on DVE
                    vb = wp.tile([128, 8, 65], bf16, tag="vb", bufs=3,
                                 name=f"vb{lt}_{eo}")
                    nc.scalar.activation(vb[:, :, 0:64],
                                         pv[:].rearrange("p (h e) -> p h e", h=8),
                                         AF.Copy)
                    nc.vector.memset(vb[:, :, 64:65], 1.0)
                    pending = (kb, vb, lt, eo)
            flush_kv(pending)
            for g in range(4):
                nc.vector.tensor_copy(kv_sb[g][:], kv_ps[g][:])

            # ---- phase 2: q, attention, output projection ---------------
            qts = {}    # (lc, h) -> q_ tile
            pans = {}   # (lc, h) -> attention rows 0:64 + denom row 64
            zbts = {}   # (lc, h) -> reciprocal denominator broadcast tiles
            ats = {}    # (lc, m) -> scaled attention lhsT tiles
            # denominators live at partitions {0,32,64,96}; the zq/zqb
            # lifetime is intra-quad now (copy -> recip -> cast -> bcast)
            # so two column sets alternating on quad parity suffice.
            # zq is fp32 for the batched reciprocal, zqb its bf16 cast
            zq = wp.tile([128, 2, LCW], f32, tag="zq")
            zqb = wp.tile([128, 2, LCW], bf16, tag="zqb")
            nc.vector.memset(zq[:], 1.0)

            def qp_group(lc, m):
                pq = pp.tile([128, LCW], f32, tag="A", bufs=2,
                             name=f"pq{m}_{lc}")
                for k in range(KT):
                    nc.tensor.matmul(pq[:], wqt[:, k, m * 128:(m + 1) * 128],
                                     xtq[lc][:, k, :],
                                     start=(k == 0), stop=(k == KT - 1))
                qr = wp.tile([128, LCW], bf16, tag="qr", bufs=2,
                             name=f"qr{m}_{lc}")
                nc.scalar.activation(qr[:], pq[:], AF.Relu)
                csl = slice(lc * LCW, (lc + 1) * LCW)
                for j in range(2):
                    h = 2 * m + j
                    q_h = wp.tile([128, LCW], bf16, tag=f"qt{h}",
                                  name=f"qt{h}_{lc}", bufs=2)
                    rows = slice(j * 64, j * 64 + 64)
                    sin_src = (srow if j == 0 else srow2)[rows, csl]
                    cos_src = (srow2 if j == 0 else srow)[rows, csl]
                    nc.vector.tensor_tensor(q_h[0:64, :], qr[rows, :],
                                            sin_src, AL.mult)
                    nc.vector.tensor_tensor(q_h[64:128, :], qr[rows, :],
                                            cos_src, AL.mult)
                    qts[(lc, h)] = q_h

            def attn_quad(lc, g):
                for h in range(4 * g, 4 * g + 4):
                    pa = pp.tile([65, LCW], f32, tag=f"kvp{h % 4}",
                                 name=f"pa{h}_{lc}")
                    nc.tensor.matmul(pa[:], kv_sb[h // 4][:, h % 4, :],
                                     qts[(lc, h)][:], start=True, stop=True)
                    # prompt evacuation frees the PSUM bank: rows 0:64 =
                    # numerator (ACT copy); denominator row 64 -> partition
                    # 32*(h%4) of the fp32 zq quad block (engine APs need
                    # 32-aligned partition bases)
                    pan = wp.tile([65, LCW], bf16, tag=f"pan{h}", bufs=1,
                                  name=f"pan{h}_{lc}")
                    nc.scalar.activation(pan[:], pa[:], AF.Copy)
                    pans[(lc, h)] = pan
                    # den row: pan row 64 -> zqb row via a tiny DMA on the
                    # idle gpsimd queue -- the ACT engine is the saturated
                    # one and a [1,512] ACT op costs as much as [65,512]
                    nc.gpsimd.dma_start(
                        zqb[32 * (h % 4):32 * (h % 4) + 1, g % 2, :],
                        pan[64:65, :])
                # batched fp32 convert + reciprocal + bf16 cast for the
                # quad, then bounce the four rows to DRAM in one strided
                # DMA (step-0 APs are illegal on SBUF but legal on DRAM)
                # and read each back broadcast to 64 partitions
                nc.vector.tensor_copy(zq[:, g % 2, :], zqb[:, g % 2, :])
                nc.vector.reciprocal_approx_fast(zq[:, g % 2, :],
                                                 zq[:, g % 2, :])
                nc.vector.tensor_copy(zqb[:, g % 2, :], zq[:, g % 2, :])
                zd4 = dp.tile([4, LCW], bf16, tag=f"zd{g % 2}", bufs=2,
                              name=f"zd{g}_{lc}")
                nc.sync.dma_start(zd4[:], zqb[0:128:32, g % 2, :])
                for h in range(4 * g, 4 * g + 4):
                    zbt = wp.tile([64, LCW], bf16, tag=f"zb{h % 8}", bufs=2,
                                  name=f"zb{h}_{lc}")
                    src = zd4[h % 4:h % 4 + 1, :]
                    src = bass.AP(tensor=src.tensor, offset=src.offset,
                                  ap=[[0, 64], [1, LCW]])
                    nc.sync.dma_start(zbt[:], src)
                    zbts[(lc, h)] = zbt

            def chain_quad(lc, g):
                if g == 0:
                    for m in range(KT):
                        ats[(lc, m)] = wp.tile([128, LCW], bf16, tag=f"at{m}",
                                               name=f"at{m}_{lc}", bufs=2)
                for h in range(4 * g, 4 * g + 4):
                    rows = slice((h % 2) * 64, (h % 2) * 64 + 64)
                    nc.vector.tensor_tensor(ats[(lc, h // 2)][rows, :],
                                            pans[(lc, h)][0:64, :],
                                            zbts[(lc, h)][:], AL.mult)

            def outp_block(lc, ltl, eo):
                lt = lc * 4 + ltl
                tsl = slice(ltl * 128, (ltl + 1) * 128)
                esl = slice(eo * 512, (eo + 1) * 512)
                po = pp.tile([128, 512], f32, tag="B", bufs=2,
                             name=f"po{lt}_{eo}")
                for m in range(KT):
                    nc.tensor.matmul(po[:], ats[(lc, m)][:, tsl],
                                     wot[:, m, esl],
                                     start=(m == 0), stop=(m == KT - 1))
                ob = wp.tile([128, 512], bf16, tag="ob", bufs=2,
                             name=f"ob{lt}_{eo}")
                nc.scalar.activation(ob[:], po[:], AF.Copy)
                nc.sync.dma_start(out_d[lt * 128:(lt + 1) * 128, esl], ob[:])

            for m in range(KT):
                qp_group(0, m)
            # w0: attn(0) quads + qp(1), then chains(0)
            for g in range(4):
                attn_quad(0, g)
                qp_group(1, 2 * g)
                qp_group(1, 2 * g + 1)
            for g in range(4):
                chain_quad(0, g)
            # w1: attn(1) quads + qp(2), then out(0), then chains(1)
            for g in range(4):
                attn_quad(1, g)
                qp_group(2, 2 * g)
                qp_group(2, 2 * g + 1)
            for ltl in range(4):
                outp_block(0, ltl, 0)
                outp_block(0, ltl, 1)
            for g in range(4):
                chain_quad(1, g)
            # w2: attn(2) quads + qp(3), then out(1), then chains(2)
            for g in range(4):
                attn_quad(2, g)
                qp_group(3, 2 * g)
                qp_group(3, 2 * g + 1)
            for ltl in range(4):
                outp_block(1, ltl, 0)
                outp_block(1, ltl, 1)
            for g in range(4):
                chain_quad(2, g)
            # w3: attn(3) quads interleaved with out(2), chains(3) inline
            for g in range(4):
                attn_quad(3, g)
                outp_block(2, g, 0)
                outp_block(2, g, 1)
                chain_quad(3, g)
            # w4: pure out(3)
            for ltl in range(4):
                outp_block(3, ltl, 0)
                outp_block(3, ltl, 1)

    nc.compile()
    _CACHE["nc"] = nc
    return nc


def _prep_inputs(query, Wq, Wk, Wv, Wo):
    idx = (np.pi / 2) * np.arange(1, L + 1, dtype=np.float64) / L
    sin = np.sin(idx).astype(np.float32)
    cos = np.cos(idx).astype(np.float32)
    # scol[p, t, c]: c=0 sin, c=1 cos at l = t*128+p
    scol = np.stack([sin.reshape(LT, 128).T, cos.reshape(LT, 128).T],
                    axis=2).reshape(128, LT * 2).copy()
    srow = np.concatenate([np.tile(sin[None, :], (64, 1)),
                           np.tile(cos[None, :], (64, 1))], axis=0).astype(BF)
    srow2 = np.concatenate([srow[64:128], srow[0:64]]).copy()

    ws = {n: np.ascontiguousarray(w.T).astype(BF)
          for n, w in (("wq", Wq), ("wk", Wk), ("wv", Wv), ("wo", Wo))}
    in_maps = []
    for b in range(NB):
        m = dict(ws)
        m["xt"] = np.ascontiguousarray(query[:, b, :].T).astype(BF)
        m["scol"] = scol
        m["srow"] = srow
        m["srow2"] = srow2
        in_maps.append(m)
    return in_maps


def kernel(query, Wq, Wk, Wv, Wo, _trace=False, _trace_kwargs=None):
    nc = _build()
    in_maps = _prep_inputs(np.asarray(query, np.float32), Wq, Wk, Wv, Wo)
    res = run_bass_kernel_spmd(nc, in_maps, list(range(NB)), trace=_trace,
                               **(_trace_kwargs or {}))
    out = np.stack([np.asarray(res.results[b]["out"]) for b in range(NB)],
                   axis=1)
    if _trace:
        kernel.last_result = res
    return np.ascontiguousarray(out.astype(np.float32))



# revision 19
# speedup vs baseline: 1.1171x; 1.0288x over previous
"""cosFormer non-causal linear attention on 8 trn2 NeuronCores.

Data-parallel over batch N=8: core b computes batch element b end-to-end.
Per core (L=2048, E=1024, H=16 heads, d=64):
  q = relu(x @ Wq.T), k = relu(x @ Wk.T), v = x @ Wv.T
  q_ = [q*sin, q*cos], k_ = [k*sin, k*cos]    (per-position cos/sin reweight)
  kv_h = k_h^T @ [v_h | 1]                     (accumulated over L in PSUM;
                                                the ones column yields k_sum)
  attn_h^T = kv_h^T @ q_h^T, rows 0:64 scaled by z = 1/row 64
  out = attn @ Wo.T

Layouts: activations contract over E_in, so x and all weights enter
transposed (host-side). k_, v live L-major; q_, attn^T live feature-major,
which feeds the output projection without any transposes on device.
All matmuls bf16 with fp32 PSUM accumulation.

Schedule notes (v4):
- Inputs arrive via a few large DMAs, all issued on the sync queue in
  first-use order so arrival order matches need order; xt comes in
  L-quarters so the first k-projection group only needs wk-half0 +
  xt-quarter0 + wv-half0 (2.5 MB) instead of 6 MB.
- A dozen dummy matmuls on zero tiles run during the DMA wait to hold the
  PE HAM clock-gate open so real matmuls start at 2.4 GHz.
- Phase 2 z-chain (denominator -> reciprocal -> partition-broadcast ->
  scale) is off the PE critical path: each attention PSUM is evacuated by
  one [65,512] ACT copy, so the 4-bank pa rotation never waits on the
  chain.  Denominator rows are assembled by ACT at 32-aligned partitions
  (engine APs require 32-aligned partition bases) so reciprocals batch 4
  heads per DVE op; reciprocal rows are then staged back to partition-0
  tiles because the GpSimd partition_broadcast ucode reads partition 0
  regardless of the AP offset (HW diverges from the simulator here).
- The output projection of chunk lc is emitted a window later
  (attn(lc)+qproj(lc+1) quads -> out(lc-1) -> z-chains(lc)), so chains
  drain under other PE work; attention head-quads interleave with
  q-projection groups so the PE never stalls on the bank rotation, and
  attn(3) interleaves with out(2) so the final window is pure out(3).
- eps is dropped from the denominator: it is a sum of 128 nonnegative
  products and is zero only if relu killed all 64 features of a query
  position (probability ~2^-64).
- Output is written bf16 (host upcasts) to halve the out-DMA bytes.

PSUM plan (8 banks): A(2 bufs): pk/pq (+warmup dummies), B(2): pv/po,
kvp0-3 (1 each): kv accumulators in phase 1, then recycled as attention
psums in phase 2.  kv slices accumulate across all of phase 1 with
start=False (banks pre-cleared by a K=1 zero matmul with stop=False:
start=True clears has_written for the WHOLE bank, so head-slices sharing
a bank never issue their own starts, and only the last head per bank
issues stop).
"""

import numpy as np
import ml_dtypes

import concourse.tile as tile
from concourse import bacc, bass, mybir
from concourse.bass_utils import run_bass_kernel_spmd

L, NB, E, H, D = 2048, 8, 1024, 16, 64
KT = E // 128          # 8 contraction tiles
LC = 4                 # L chunks of 512
LCW = L // LC          # 512
LT = L // 128          # 16 l-tiles
NWARM = 16             # HAM warm-up dummy matmuls

f32 = mybir.dt.float32
bf16 = mybir.dt.bfloat16
AL = mybir.AluOpType
AF = mybir.ActivationFunctionType
BF = np.dtype(ml_dtypes.bfloat16)

_CACHE = {}


def _build():
    if "nc" in _CACHE:
        return _CACHE["nc"]
    nc = bacc.Bacc()

    xt_d = nc.declare_dram_parameter("xt", [E, L], bf16, isOutput=False)
    wq_d = nc.declare_dram_parameter("wq", [E, E], bf16, isOutput=False)
    wk_d = nc.declare_dram_parameter("wk", [E, E], bf16, isOutput=False)
    wv_d = nc.declare_dram_parameter("wv", [E, E], bf16, isOutput=False)
    wo_d = nc.declare_dram_parameter("wo", [E, E], bf16, isOutput=False)
    scol_d = nc.declare_dram_parameter("scol", [128, LT * 2], f32, isOutput=False)
    srow_d = nc.declare_dram_parameter("srow", [128, L], bf16, isOutput=False)
    srow2_d = nc.declare_dram_parameter("srow2", [128, L], bf16, isOutput=False)
    out_d = nc.declare_dram_parameter("out", [L, E], bf16, isOutput=True)

    with tile.TileContext(nc) as tc:
        with (
            tc.tile_pool(name="const", bufs=1) as cp,
            tc.tile_pool(name="work", bufs=1) as wp,
            tc.tile_pool(name="ps", bufs=1, space="PSUM") as pp,
            tc.tile_pool(name="dscratch", bufs=1, space="DRAM") as dp,
        ):
            # ---- resident constants -------------------------------------
            # Big DMAs, all on the sync queue, in first-use order.
            xt_re = xt_d.rearrange("(t p) l -> p t l", p=128)
            w_re = {n: d.rearrange("(t p) e -> p t e", p=128)
                    for n, d in (("wk", wk_d), ("wv", wv_d), ("wq", wq_d),
                                 ("wo", wo_d))}
            scol = cp.tile([128, LT, 2], f32, tag="scol")
            wkh = [cp.tile([128, KT, 512], bf16, tag=f"wkh{h}", name=f"wkh{h}")
                   for h in range(2)]
            wvh = [cp.tile([128, KT, 512], bf16, tag=f"wvh{h}", name=f"wvh{h}")
                   for h in range(2)]
            xtq = [cp.tile([128, KT, 512], bf16, tag=f"xtq{q}", name=f"xtq{q}")
                   for q in range(4)]
            wqt = cp.tile([128, KT, E], bf16, tag="wqt")
            wot = cp.tile([128, KT, E], bf16, tag="wot")
            # srow rows 0:64 = sin (replicated), rows 64:128 = cos;
            # srow2 is srow with halves swapped (TensorTensor requires
            # equal base partitions when both inputs are in SBUF)
            srow = cp.tile([128, L], bf16, tag="srow")
            srow2 = cp.tile([128, L], bf16, tag="srow2")

            # first-need tensors split across the two HWDGE rings: the
            # scalar (qActDynamicHW) queue clears its startup barrier ~2us
            # before sync does, and the two rings issue in parallel
            nc.scalar.dma_start(scol[:], scol_d.rearrange("p (t c) -> p t c", c=2)[:])
            nc.scalar.dma_start(wkh[0][:], w_re["wk"][:, :, 0:512])
            nc.scalar.dma_start(wvh[0][:], w_re["wv"][:, :, 0:512])
            nc.sync.dma_start(xtq[0][:], xt_re[:, :, 0:512])
            nc.sync.dma_start(wkh[1][:], w_re["wk"][:, :, 512:1024])
            nc.sync.dma_start(wvh[1][:], w_re["wv"][:, :, 512:1024])
            for q in range(1, 4):
                nc.sync.dma_start(xtq[q][:], xt_re[:, :, q * 512:(q + 1) * 512])
            nc.sync.dma_start(wqt[:], w_re["wq"][:])
            nc.sync.dma_start(wot[:], w_re["wo"][:])
            nc.sync.dma_start(srow[:], srow_d[:])
            nc.sync.dma_start(srow2[:], srow2_d[:])

            # kv lhsT tiles in SBUF: 4 groups of 4 heads, (128=2d, 4, 65)
            kv_sb = [wp.tile([128, 4, 65], bf16, tag=f"kv{g}", name=f"kv{g}")
                     for g in range(4)]

            # ---- psum accumulators for phase 1 --------------------------
            kv_ps = [pp.tile([128, 4, 65], f32, tag=f"kvp{g}", name=f"kvp{g}")
                     for g in range(4)]
            zl = wp.tile([1, 128], bf16, tag="zl")
            zrow = wp.tile([1, 512], bf16, tag="zrow")
            nc.vector.memset(zl[:], 0.0)
            nc.vector.memset(zrow[:], 0.0)
            for g in range(4):
                nc.tensor.matmul(kv_ps[g][:].rearrange("p a c -> p (a c)"),
                                 zl[:], zrow[:, 0:260], start=True, stop=False)
            # HAM warm-up: keep PE busy during the input DMA wait so real
            # matmuls start at the full clock.  Never read.
            dmy = pp.tile([128, 512], f32, tag="A", bufs=2, name="dmy")
            for _ in range(NWARM):
                nc.tensor.matmul(dmy[:], zl[:], zrow[:], start=True, stop=True)

            # ---- phase 1: kv accumulation (kv mms pipelined 1 iter back,
            # so PE has a full projection group between an ACT build and its
            # kv consumers and never stalls on ACT latency) ---------------
            pending = None
            def flush_kv(pend):
                kb_, vb_, lt_, eo_ = pend
                for hh in range(8):
                    h = eo_ * 8 + hh
                    nc.tensor.matmul(kv_ps[h // 4][:, h % 4, :],
                                     kb_[:, hh, :], vb_[:, hh, :],
                                     start=False,
                                     stop=(lt_ == LT - 1 and hh % 4 == 3))
            for lt in range(LT):
                q4, off = divmod(lt, 4)
                lsl = slice(off * 128, off * 128 + 128)
                for eo in range(2):
                    pk = pp.tile([128, 512], f32, tag="A", bufs=2, name=f"pk{lt}_{eo}")
                    pv = pp.tile([128, 512], f32, tag="B", bufs=2, name=f"pv{lt}_{eo}")
                    for k in range(KT):
                        nc.tensor.matmul(pk[:], xtq[q4][:, k, lsl], wkh[eo][:, k, :],
                                         start=(k == 0), stop=(k == KT - 1))
                    if pending is not None:
                        flush_kv(pending)
                    for k in range(KT):
                        nc.tensor.matmul(pv[:], xtq[q4][:, k, lsl], wvh[eo][:, k, :],
                                         start=(k == 0), stop=(k == KT - 1))
                    # k_ build on ACT: per head [0:64]=relu(k)*sin, [64:128]=relu(k)*cos
                    kb = wp.tile([128, 8, 128], bf16, tag="kb", bufs=3,
                                 name=f"kb{lt}_{eo}")
                    pk3 = pk[:].rearrange("p (h e) -> p h e", h=8)
                    nc.scalar.activation(kb[:, :, 0:64], pk3,
                                         AF.Relu, scale=scol[:, lt, 0:1])
                    nc.scalar.activation(kb[:, :, 64:128], pk3,
                                         AF.Relu, scale=scol[:, lt, 1:2])
                    # v copy on ACT into 65-wide layout; ones col on DVE
                    vb = wp.tile([128, 8, 65], bf16, tag="vb", bufs=3,
                                 name=f"vb{lt}_{eo}")
                    nc.scalar.activation(vb[:, :, 0:64],
                                         pv[:].rearrange("p (h e) -> p h e", h=8),
                                         AF.Copy)
                    nc.vector.memset(vb[:, :, 64:65], 1.0)
                    pending = (kb, vb, lt, eo)
            # the final kv flush + evacuation is emitted below, interleaved
            # with the first q-projection groups so the PE never idles
            # across the phase boundary

            # ---- phase 2: q, attention, output projection ---------------
            qts = {}    # (lc, h) -> q_ tile
            pans = {}   # (lc, h) -> attention rows 0:64 + denom row 64
            zbts = {}   # (lc, h) -> reciprocal denominator broadcast tiles
            ats = {}    # (lc, m) -> scaled attention lhsT tiles
            # denominators live at partitions {0,32,64,96}; the zq/zqb
            # lifetime is intra-quad now (copy -> recip -> cast -> bcast)
            # so two column sets alternating on quad parity suffice.
            # zq is fp32 for the batched reciprocal, zqb its bf16 cast
            zq = wp.tile([128, 2, LCW], f32, tag="zq")
            zqb = wp.tile([128, 2, LCW], bf16, tag="zqb")
            nc.vector.memset(zq[:], 1.0)

            def qp_group(lc, m):
                pq = pp.tile([128, LCW], f32, tag="A", bufs=2,
                             name=f"pq{m}_{lc}")
                for k in range(KT):
                    nc.tensor.matmul(pq[:], wqt[:, k, m * 128:(m + 1) * 128],
                                     xtq[lc][:, k, :],
                                     start=(k == 0), stop=(k == KT - 1))
                qr = wp.tile([128, LCW], bf16, tag="qr", bufs=2,
                             name=f"qr{m}_{lc}")
                nc.scalar.activation(qr[:], pq[:], AF.Relu)
                csl = slice(lc * LCW, (lc + 1) * LCW)
                for j in range(2):
                    h = 2 * m + j
                    q_h = wp.tile([128, LCW], bf16, tag=f"qt{h}",
                                  name=f"qt{h}_{lc}", bufs=2)
                    rows = slice(j * 64, j * 64 + 64)
                    sin_src = (srow if j == 0 else srow2)[rows, csl]
                    cos_src = (srow2 if j == 0 else srow)[rows, csl]
                    nc.vector.tensor_tensor(q_h[0:64, :], qr[rows, :],
                                            sin_src, AL.mult)
                    nc.vector.tensor_tensor(q_h[64:128, :], qr[rows, :],
                                            cos_src, AL.mult)
                    qts[(lc, h)] = q_h

            def attn_quad(lc, g):
                for h in range(4 * g, 4 * g + 4):
                    pa = pp.tile([65, LCW], f32, tag=f"kvp{h % 4}",
                                 name=f"pa{h}_{lc}")
                    nc.tensor.matmul(pa[:], kv_sb[h // 4][:, h % 4, :],
                                     qts[(lc, h)][:], start=True, stop=True)
                    # prompt evacuation frees the PSUM bank: rows 0:64 =
                    # numerator (ACT copy); denominator row 64 -> partition
                    # 32*(h%4) of the fp32 zq quad block (engine APs need
                    # 32-aligned partition bases)
                    pan = wp.tile([65, LCW], bf16, tag=f"pan{h}", bufs=1,
                                  name=f"pan{h}_{lc}")
                    nc.scalar.activation(pan[:], pa[:], AF.Copy)
                    pans[(lc, h)] = pan
                    # den row: pan row 64 -> zqb row via a tiny DMA on the
                    # idle gpsimd queue -- the ACT engine is the saturated
                    # one and a [1,512] ACT op costs as much as [65,512]
                    nc.gpsimd.dma_start(
                        zqb[32 * (h % 4):32 * (h % 4) + 1, g % 2, :],
                        pan[64:65, :])
                # batched fp32 convert + reciprocal + bf16 cast for the
                # quad, then bounce the four rows to DRAM in one strided
                # DMA (step-0 APs are illegal on SBUF but legal on DRAM)
                # and read each back broadcast to 64 partitions
                nc.vector.tensor_copy(zq[:, g % 2, :], zqb[:, g % 2, :])
                nc.vector.reciprocal_approx_fast(zq[:, g % 2, :],
                                                 zq[:, g % 2, :])
                nc.vector.tensor_copy(zqb[:, g % 2, :], zq[:, g % 2, :])
                zd4 = dp.tile([4, LCW], bf16, tag=f"zd{g % 2}", bufs=2,
                              name=f"zd{g}_{lc}")
                nc.sync.dma_start(zd4[:], zqb[0:128:32, g % 2, :])
                for h in range(4 * g, 4 * g + 4):
                    zbt = wp.tile([64, LCW], bf16, tag=f"zb{h % 8}", bufs=2,
                                  name=f"zb{h}_{lc}")
                    src = zd4[h % 4:h % 4 + 1, :]
                    src = bass.AP(tensor=src.tensor, offset=src.offset,
                                  ap=[[0, 64], [1, LCW]])
                    nc.sync.dma_start(zbt[:], src)
                    zbts[(lc, h)] = zbt

            def chain_quad(lc, g):
                if g == 0:
                    for m in range(KT):
                        ats[(lc, m)] = wp.tile([128, LCW], bf16, tag=f"at{m}",
                                               name=f"at{m}_{lc}", bufs=2)
                for h in range(4 * g, 4 * g + 4):
                    rows = slice((h % 2) * 64, (h % 2) * 64 + 64)
                    nc.vector.tensor_tensor(ats[(lc, h // 2)][rows, :],
                                            pans[(lc, h)][0:64, :],
                                            zbts[(lc, h)][:], AL.mult)

            def outp_block(lc, ltl, eo):
                lt = lc * 4 + ltl
                tsl = slice(ltl * 128, (ltl + 1) * 128)
                esl = slice(eo * 512, (eo + 1) * 512)
                po = pp.tile([128, 512], f32, tag="B", bufs=2,
                             name=f"po{lt}_{eo}")
                for m in range(KT):
                    nc.tensor.matmul(po[:], ats[(lc, m)][:, tsl],
                                     wot[:, m, esl],
                                     start=(m == 0), stop=(m == KT - 1))
                ob = wp.tile([128, 512], bf16, tag="ob", bufs=2,
                             name=f"ob{lt}_{eo}")
                nc.scalar.activation(ob[:], po[:], AF.Copy)
                nc.sync.dma_start(out_d[lt * 128:(lt + 1) * 128, esl], ob[:])

            # phase boundary: first q-projection groups interleave with the
            # final kv flush + psum evacuation so the PE stays fed
            qp_group(0, 0)
            flush_kv(pending)
            for g in range(4):
                nc.vector.tensor_copy(kv_sb[g][:], kv_ps[g][:])
            qp_group(0, 1)
            for m in range(2, KT):
                qp_group(0, m)
            # chains for quads (lc, g) are emitted ~2 quads later -- deep
            # enough that the z-broadcast DRAM round trip has landed, so a
            # chain TT never blocks the DVE queue head and delays the q~
            # builds behind it
            # w0: attn(0) quads + qp(1); chains(0,0..1) late in window
            for g in range(4):
                attn_quad(0, g)
                if g == 2:
                    chain_quad(0, 0)
                if g == 3:
                    chain_quad(0, 1)
                qp_group(1, 2 * g)
                qp_group(1, 2 * g + 1)
            # w1: attn(1) + qp(2) + chains(0,2..3)/(1,0..1), then out(0)
            for g, (clc, cg) in enumerate([(0, 2), (0, 3), (1, 0), (1, 1)]):
                attn_quad(1, g)
                chain_quad(clc, cg)
                qp_group(2, 2 * g)
                qp_group(2, 2 * g + 1)
            for ltl in range(4):
                outp_block(0, ltl, 0)
                outp_block(0, ltl, 1)
            # w2: attn(2) + qp(3) + chains(1,2..3)/(2,0..1), then out(1)
            for g, (clc, cg) in enumerate([(1, 2), (1, 3), (2, 0), (2, 1)]):
                attn_quad(2, g)
                chain_quad(clc, cg)
                qp_group(3, 2 * g)
                qp_group(3, 2 * g + 1)
            for ltl in range(4):
                outp_block(1, ltl, 0)
                outp_block(1, ltl, 1)
            # w3: attn(3) quads front-loaded between out(2) pairs; chains
            # (2,2..3) first so out(2) has all its ats, then (3,0..1)
            attn_quad(3, 0)
            chain_quad(2, 2)
            chain_quad(2, 3)
            outp_block(2, 0, 0)
            outp_block(2, 0, 1)
            attn_quad(3, 1)
            outp_block(2, 1, 0)
            outp_block(2, 1, 1)
            attn_quad(3, 2)
            chain_quad(3, 0)
            outp_block(2, 2, 0)
            outp_block(2, 2, 1)
            attn_quad(3, 3)
            chain_quad(3, 1)
            outp_block(2, 3, 0)
            outp_block(2, 3, 1)
            # w4: last chains then pure out(3)
            chain_quad(3, 2)
            chain_quad(3, 3)
            for ltl in range(4):
                outp_block(3, ltl, 0)
                outp_block(3, ltl, 1)

    nc.compile()
    _CACHE["nc"] = nc
    return nc


def _prep_inputs(query, Wq, Wk, Wv, Wo):
    idx = (np.pi / 2) * np.arange(1, L + 1, dtype=np.float64) / L
    sin = np.sin(idx).astype(np.float32)
    cos = np.cos(idx).astype(np.float32)
    # scol[p, t, c]: c=0 sin, c=1 cos at l = t*128+p
    scol = np.stack([sin.reshape(LT, 128).T, cos.reshape(LT, 128).T],
                    axis=2).reshape(128, LT * 2).copy()
    srow = np.concatenate([np.tile(sin[None, :], (64, 1)),
                           np.tile(cos[None, :], (64, 1))], axis=0).astype(BF)
    srow2 = np.concatenate([srow[64:128], srow[0:64]]).copy()

    ws = {n: np.ascontiguousarray(w.T).astype(BF)
          for n, w in (("wq", Wq), ("wk", Wk), ("wv", Wv), ("wo", Wo))}
    in_maps = []
    for b in range(NB):
        m = dict(ws)
        m["xt"] = np.ascontiguousarray(query[:, b, :].T).astype(BF)
        m["scol"] = scol
        m["srow"] = srow
        m["srow2"] = srow2
        in_maps.append(m)
    return in_maps


def kernel(query, Wq, Wk, Wv, Wo, _trace=False, _trace_kwargs=None):
    nc = _build()
    in_maps = _prep_inputs(np.asarray(query, np.float32), Wq, Wk, Wv, Wo)
    res = run_bass_kernel_spmd(nc, in_maps, list(range(NB)), trace=_trace,
                               **(_trace_kwargs or {}))
    out = np.stack([np.asarray(res.results[b]["out"]) for b in range(NB)],
                   axis=1)
    if _trace:
        kernel.last_result = res
    return np.ascontiguousarray(out.astype(np.float32))

